# revision 1
# baseline (speedup 1.0000x reference)
"""Trainium2 Bass kernel for a 4-layer transformer (B=2,S=1024,D=1024,H=16,F=4096,V=32000).

Strategy (8 NeuronCores):
 - Sequence-parallel transformer layers: each core owns 256 tokens
   (cores 0-3: batch 0, cores 4-7: batch 1). All layer weights replicated
   (streamed from HBM as bf16). Activations kept feature-major
   ([d on partitions, tokens on free]) so no transposes are needed in the
   layer loop; per-token stats (LN mean/var, softmax 1/Z) are computed with
   ones-matmuls and broadcast back with K=1 matmuls.
 - Attention: per-layer AllGather of K/V (bf16) within each batch's 4-core
   group; scores computed transposed (s^T[kt,q]) so softmax-weighted sums
   contract on the partition axis without transposing P.
 - LM head: final LN output AllGather-ed across all 8 cores; each core
   computes a 4000-wide vocab shard of the logits for all 2048 tokens.

Self-contained: hardcodes all shapes; host side only reshapes/shards/casts.
"""
import numpy as np
import ml_dtypes

import concourse.bass as bass
import concourse.bacc as bacc
import concourse.mybir as mybir
import concourse.tile as tile
from concourse import bass_utils
from concourse.masks import make_identity

B, S, D, H, L, F, V = 2, 1024, 1024, 16, 4, 4096, 32000
DH = D // H          # 64
NCORES = 8
T = (B * S) // NCORES  # 256 tokens per core
NT = B * S             # 2048
VS = V // NCORES       # 4000
VSP = 4096             # padded vocab shard
P = 128
ND = D // P            # 8 d-tiles
NFT = F // P           # 32 fc1 f-tiles

f32 = mybir.dt.float32
bf16 = mybir.dt.bfloat16
i32 = mybir.dt.int32
AF = mybir.ActivationFunctionType
OP = mybir.AluOpType


def _ln(nc, ps, act, rows, cons, x_sb, s_t, b_t, out_h, ident_unused=None):
    """LayerNorm: x_sb [128, 8, 256] f32 -> out_h [128, 8, 256] bf16.

    s_t/b_t: [128, 8] f32 param tiles (column dt = scale/bias for features
    dt*128..dt*128+127)."""
    stat_x = ps.tile([1, 256], f32, tag="att_s", bufs=4)
    stat_sq = ps.tile([1, 256], f32, tag="att_s", bufs=4)
    for dt in range(ND):
        sq = act.tile([P, 256], f32, tag="sq")
        nc.vector.tensor_mul(sq[:], x_sb[:, dt, :], x_sb[:, dt, :])
        nc.tensor.matmul(stat_x[:], lhsT=cons.ones_col_f32[:], rhs=x_sb[:, dt, :],
                         start=(dt == 0), stop=(dt == ND - 1))
        nc.tensor.matmul(stat_sq[:], lhsT=cons.ones_col_f32[:], rhs=sq[:],
                         start=(dt == 0), stop=(dt == ND - 1))
    murow = rows.tile([1, 512], f32, tag="row")  # [mu | invstd]
    nc.scalar.activation(murow[:, 0:256], stat_x[:], AF.Copy, scale=1.0 / D)
    msq = rows.tile([1, 256], f32, tag="row")
    nc.scalar.activation(msq[:], stat_sq[:], AF.Copy, scale=1.0 / D)
    var = rows.tile([1, 256], f32, tag="row")
    nc.vector.tensor_mul(var[:], murow[:, 0:256], murow[:, 0:256])
    nc.vector.tensor_sub(var[:], msq[:], var[:])
    std = rows.tile([1, 256], f32, tag="row")
    nc.scalar.activation(std[:], var[:], AF.Sqrt, bias=cons.eps_row[:, 0:1])
    nc.vector.reciprocal(murow[:, 256:512], std[:])
    bc = act.tile([P, 512], f32, tag="lnbc", bufs=2)
    nc.gpsimd.partition_broadcast(bc[:], murow[:], channels=P)
    for dt in range(ND):
        t = act.tile([P, 256], f32, tag="sq")
        nc.vector.tensor_sub(t[:], x_sb[:, dt, :], bc[:, 0:256])
        nc.vector.tensor_mul(t[:], t[:], bc[:, 256:512])
        nc.vector.tensor_scalar(out_h[:, dt, :], t[:], s_t[:, dt:dt + 1],
                                b_t[:, dt:dt + 1], OP.mult, OP.add)


class _Cons:
    pass


def build(n_layers=L, single=False):
    """single=True: 1-core variant with collectives replaced by local DMA
    copies (for TimelineSim cost-model analysis only — wrong numerics)."""
    nc = bacc.Bacc("TRN2", target_bir_lowering=False, debug=False,
                   num_devices=1 if single else NCORES)

    ids = nc.dram_tensor("ids", [T], i32, kind="ExternalInput").ap()
    pos = nc.dram_tensor("pos", [T, D], f32, kind="ExternalInput").ap()
    embed_w = nc.dram_tensor("embed_w", [V, D], f32, kind="ExternalInput").ap()
    attn_wT = nc.dram_tensor("attn_wT", [L, D, 3 * D], bf16, kind="ExternalInput").ap()
    attn_in_b = nc.dram_tensor("attn_in_b", [L, 3 * D], f32, kind="ExternalInput").ap()
    proj_wT = nc.dram_tensor("proj_wT", [L, D, D], bf16, kind="ExternalInput").ap()
    proj_b = nc.dram_tensor("proj_b", [L, D], f32, kind="ExternalInput").ap()
    fc1_wT = nc.dram_tensor("fc1_wT", [L, D, F], bf16, kind="ExternalInput").ap()
    fc1_b = nc.dram_tensor("fc1_b", [L, F], f32, kind="ExternalInput").ap()
    fc2_wT = nc.dram_tensor("fc2_wT", [L, F, D], bf16, kind="ExternalInput").ap()
    fc2_b = nc.dram_tensor("fc2_b", [L, D], f32, kind="ExternalInput").ap()
    ln1_s = nc.dram_tensor("ln1_s", [L, D], f32, kind="ExternalInput").ap()
    ln1_b = nc.dram_tensor("ln1_b", [L, D], f32, kind="ExternalInput").ap()
    ln2_s = nc.dram_tensor("ln2_s", [L, D], f32, kind="ExternalInput").ap()
    ln2_b = nc.dram_tensor("ln2_b", [L, D], f32, kind="ExternalInput").ap()
    lnf_s = nc.dram_tensor("lnf_s", [D], f32, kind="ExternalInput").ap()
    lnf_b = nc.dram_tensor("lnf_b", [D], f32, kind="ExternalInput").ap()
    lm_wT = nc.dram_tensor("lm_wT", [D, VSP], bf16, kind="ExternalInput").ap()
    lm_b = nc.dram_tensor("lm_b", [VSP], f32, kind="ExternalInput").ap()
    out_tok = nc.dram_tensor("out_tok", [NT, VSP], f32, kind="ExternalOutput").ap()

    kv_groups = [[0, 1, 2, 3], [4, 5, 6, 7]]
    all_group = [list(range(NCORES))]

    with tile.TileContext(nc) as tc:
        with (
            tc.tile_pool(name="consp", bufs=1) as consp,
            tc.tile_pool(name="wpool", bufs=9) as wpool,
            tc.tile_pool(name="rows", bufs=6) as rows,
            tc.tile_pool(name="par", bufs=2) as par,
            tc.tile_pool(name="dram", bufs=1, space="DRAM") as dram,
        ):
            # layer-phase pools, released before the LM phase so the LM
            # phase can use all 8 PSUM banks and the freed SBUF
            act = tc.alloc_tile_pool(name="act", bufs=1)
            ps = tc.alloc_tile_pool(name="ps", bufs=2, space="PSUM")
            cons = _Cons()
            ident = consp.tile([P, P], f32)
            make_identity(nc, ident)
            ident_bf = consp.tile([P, P], bf16)
            nc.vector.tensor_copy(ident_bf[:], ident[:])
            ones_col_f32 = consp.tile([P, 1], f32)
            nc.vector.memset(ones_col_f32[:], 1.0)
            cons.ones_col_f32 = ones_col_f32
            eps_row = consp.tile([1, 1], f32)
            nc.vector.memset(eps_row[:], 1e-5)
            cons.eps_row = eps_row

            x_sb = consp.tile([P, ND, 256], f32)  # residual, feature-major

            # per-core group-rank registers for own-block-skipping dynamic DMAs
            seng = nc.sync
            pid = seng.partition_id()
            rgrp = seng.alloc_register("grp_rank")
            seng.reg_alu(rgrp, pid, 3, OP.bitwise_and)
            grp_rank = seng.snap(rgrp, donate=True, min_val=0, max_val=3)
            oth_ranks = []
            for i in range(3):
                ra = seng.alloc_register(f"oth{i}a")
                seng.reg_alu(ra, grp_rank, i + 1, OP.add)
                rb = seng.alloc_register(f"oth{i}b")
                seng.reg_alu(rb, ra, 3, OP.bitwise_and)
                oth_ranks.append(seng.snap(rb, donate=True, min_val=0, max_val=3))

            # ---------------- embedding ----------------
            for tc2 in range(2):
                ids_sb = par.tile([P, 1], i32, tag="ids")
                nc.sync.dma_start(ids_sb[:], ids[tc2 * P:(tc2 + 1) * P, None])
                gat = wpool.tile([P, D], f32, tag="w")
                nc.gpsimd.indirect_dma_start(
                    out=gat[:], out_offset=None, in_=embed_w[:],
                    in_offset=bass.IndirectOffsetOnAxis(ap=ids_sb[:, :1], axis=0))
                pos_sb = wpool.tile([P, D], f32, tag="w")
                nc.sync.dma_start(pos_sb[:], pos[tc2 * P:(tc2 + 1) * P, :])
                nc.vector.tensor_add(gat[:], gat[:], pos_sb[:])
                for dt in range(ND):
                    tp = ps.tile([P, P], f32, tag="att_s", bufs=4)
                    nc.tensor.transpose(tp[:], gat[:, dt * P:(dt + 1) * P], ident[:])
                    nc.vector.tensor_copy(x_sb[:, dt, tc2 * P:(tc2 + 1) * P], tp[:])

            # ---------------- layers ----------------
            for l in range(n_layers):
                # LN1
                ln1s_t = par.tile([P, ND], f32, tag="lnp")
                nc.sync.dma_start(ln1s_t[:], ln1_s[l].rearrange("(k p) -> p k", p=P))
                ln1b_t = par.tile([P, ND], f32, tag="lnp")
                nc.sync.dma_start(ln1b_t[:], ln1_b[l].rearrange("(k p) -> p k", p=P))
                h_sb = act.tile([P, ND, 256], bf16, tag="h", bufs=2)
                _ln(nc, ps, act, rows, cons, x_sb, ln1s_t, ln1b_t, h_sb)

                # QKV weights: 8 d-slices of [128, 3072]
                w_qkv = []
                for dt in range(ND):
                    wt = wpool.tile([P, 4096], bf16, tag="w", name=f"wqkv{l}_{dt}")
                    nc.sync.dma_start(wt[:, 0:3 * D], attn_wT[l, dt * P:(dt + 1) * P, :])
                    w_qkv.append(wt)
                qkvb_t = par.tile([P, 24], f32, tag="qkvb")
                nc.sync.dma_start(qkvb_t[:], attn_in_b[l].rearrange("(k p) -> p k", p=P))

                q_all = act.tile([P, 8, 256], bf16, tag="q")
                k_loc = act.tile([P, 8, 256], bf16, tag="kloc")
                # K first so the AllGather can start as early as possible
                for ft in range(8, 16):
                    acc = ps.tile([P, 256], f32, tag="acc")
                    for dt in range(ND):
                        nc.tensor.matmul(acc[:], lhsT=w_qkv[dt][:, ft * P:(ft + 1) * P],
                                         rhs=h_sb[:, dt, :],
                                         start=(dt == 0), stop=(dt == ND - 1))
                    nc.scalar.activation(k_loc[:, ft - 8, :], acc[:], AF.Identity,
                                         bias=qkvb_t[:, ft:ft + 1])
                # K bounce + AllGather immediately (overlaps V/Q projections)
                k_in = dram.tile([8, P, 256], bf16, tag="kin", name=f"kin{l}")
                k_out = dram.tile([4, 8, P, 256], bf16, tag="kout", name=f"kout{l}")
                nc.sync.dma_start(k_in.rearrange("f p t -> p f t"), k_loc[:])
                if single:
                    nc.sync.dma_start(k_out[0], k_in[:])
                else:
                    nc.gpsimd.collective_compute(
                        "AllGather", OP.bypass, replica_groups=kv_groups,
                        ins=[k_in.opt()], outs=[k_out.opt()])

                # V (token-major with interleaved ones columns: per head 65
                # cols = [v_h | 1]), then V AllGather in that layout
                v_loc = act.tile([P, 2, 16 * 65], bf16, tag="vloc")
                v_loc_h = v_loc.rearrange("p c (h g) -> p c h g", h=16, g=65)
                for tc2 in range(2):
                    for nb in range(2):
                        acc = ps.tile([P, 512], f32, tag="acc")
                        for dt in range(ND):
                            nc.tensor.matmul(
                                acc[:], lhsT=h_sb[:, dt, tc2 * P:(tc2 + 1) * P],
                                rhs=w_qkv[dt][:, 2 * D + nb * 512:2 * D + (nb + 1) * 512],
                                start=(dt == 0), stop=(dt == ND - 1))
                        nc.scalar.activation(
                            v_loc_h[:, tc2, nb * 8:(nb + 1) * 8, 0:64],
                            acc[:].rearrange("p (h g) -> p h g", h=8), AF.Copy)
                    nc.vector.memset(v_loc_h[:, tc2, :, 64:65], 1.0)
                v_in = dram.tile([256, 16 * 65], bf16, tag="vin", name=f"vin{l}")
                v_out = dram.tile([4, 256, 16 * 65], bf16, tag="vout", name=f"vout{l}")
                for tc2 in range(2):
                    nc.sync.dma_start(v_in[tc2 * P:(tc2 + 1) * P, :],
                                      v_loc[:, tc2, :])
                if single:
                    nc.sync.dma_start(v_out[0], v_in[:])
                else:
                    nc.gpsimd.collective_compute(
                        "AllGather", OP.bypass, replica_groups=kv_groups,
                        ins=[v_in.opt()], outs=[v_out.opt()])

                # Q projections (overlap the AllGathers)
                for ft in range(8):
                    acc = ps.tile([P, 256], f32, tag="acc")
                    for dt in range(ND):
                        nc.tensor.matmul(acc[:], lhsT=w_qkv[dt][:, ft * P:(ft + 1) * P],
                                         rhs=h_sb[:, dt, :],
                                         start=(dt == 0), stop=(dt == ND - 1))
                    nc.scalar.activation(q_all[:, ft, :], acc[:], AF.Identity,
                                         bias=qkvb_t[:, ft:ft + 1])

                o_sb = act.tile([P, ND, 256], bf16, tag="o")
                scale = 1.0 / np.sqrt(DH)

                # Pass 1 (pre-AllGather): attention over this core's OWN 256
                # k-tokens from local k_loc/v_loc; partial [o|Z] snapshotted
                # to SBUF so the PSUM slot frees immediately.
                o_own = {}
                for j in range(8):
                    for hh in range(2):
                        h_idx = 2 * j + hh
                        base = hh * 64
                        avp = ps.tile([P, 256], f32, tag="av",
                                      name=f"avp{l}_{j}_{hh}")
                        for c in range(2):
                            sps = ps.tile([P, 256], f32, tag="att_s", bufs=4,
                                          name=f"spp{l}_{j}_{c}_{hh}")
                            nc.tensor.matmul(
                                sps[:],
                                lhsT=k_loc[base:base + 64, j, c * P:(c + 1) * P],
                                rhs=q_all[base:base + 64, j, :],
                                start=True, stop=True)
                            e = act.tile([P, 256], bf16, tag="e", bufs=8,
                                         name=f"ep{l}_{j}_{c}_{hh}")
                            nc.scalar.activation(e[:], sps[:], AF.Exp, scale=scale)
                            nc.tensor.matmul(
                                avp[0:65, :],
                                lhsT=v_loc_h[:, c, h_idx, :],
                                rhs=e[:], start=(c == 0), stop=(c == 1))
                        snap = act.tile([65, 256], bf16, tag="avown", bufs=16,
                                        name=f"oo{l}_{j}_{hh}")
                        nc.scalar.activation(snap[:], avp[0:65, :], AF.Copy)
                        o_own[(j, hh)] = snap

                # Load the three OTHER ranks' K/V blocks (partition-id-derived
                # dynamic offsets skip our own block).
                k_sb = act.tile([P, 8, 768], bf16, tag="ksb")
                for j in range(8):
                    for i in range(3):
                        nc.sync.dma_start(
                            k_sb[:, j, i * 256:(i + 1) * 256],
                            k_out[bass.ds(oth_ranks[i], 1), j, :, :].rearrange(
                                "o p t -> p (o t)"))
                v_sb = act.tile([P, 6, 16 * 65], bf16, tag="vsb")
                for i in range(3):
                    nc.sync.dma_start(
                        v_sb[:, 2 * i:2 * i + 2, :],
                        v_out[bass.ds(oth_ranks[i], 1), :, :].rearrange(
                            "o (th p) f -> p (o th) f", p=P))
                v_sb_h = v_sb.rearrange("p c (h g) -> p c h g", h=16, g=65)

                # Pass 2: re-inject the partial [o|Z] (identity matmul) and
                # accumulate the 6 remaining k-chunks; head pairs interleaved
                # so LDWEIGHTS of one head overlaps the matmul of the other.
                for j in range(8):
                    av0 = ps.tile([P, 256], f32, tag="av", name=f"av{l}_{j}_0")
                    av1 = ps.tile([P, 256], f32, tag="av", name=f"av{l}_{j}_1")
                    avs = [av0, av1]
                    for hh in range(2):
                        nc.tensor.matmul(avs[hh][0:65, :],
                                         lhsT=ident_bf[0:65, 0:65],
                                         rhs=o_own[(j, hh)][:],
                                         start=True, stop=False)
                    for c in range(6):
                        es = []
                        for hh in range(2):
                            base = hh * 64
                            sps = ps.tile([P, 256], f32, tag="att_s", bufs=4,
                                          name=f"sps{l}_{j}_{c}_{hh}")
                            nc.tensor.matmul(
                                sps[:], lhsT=k_sb[base:base + 64, j, c * P:(c + 1) * P],
                                rhs=q_all[base:base + 64, j, :], start=True, stop=True)
                            e = act.tile([P, 256], bf16, tag="e", bufs=8,
                                         name=f"e{l}_{j}_{c}_{hh}")
                            nc.scalar.activation(e[:], sps[:], AF.Exp, scale=scale)
                            es.append(e)
                        for hh in range(2):
                            h_idx = 2 * j + hh
                            # av rows 0:64 = unnormalized o, row 64 = Z
                            nc.tensor.matmul(
                                avs[hh][0:65, :],
                                lhsT=v_sb_h[:, c, h_idx, :],
                                rhs=es[hh][:], start=False, stop=(c == 5))
                    for hh in range(2):
                        recip = rows.tile([1, 256], f32, tag="row")
                        nc.vector.reciprocal(recip[:], avs[hh][64:65, :])
                        bc_sb = act.tile([P, 256], f32, tag="bcsb", bufs=2)
                        nc.gpsimd.partition_broadcast(bc_sb[0:64, :], recip[:],
                                                      channels=64)
                        if hh == 0:
                            nc.vector.tensor_mul(o_sb[0:64, j, :], avs[hh][0:64, :],
                                                 bc_sb[0:64, :])
                        else:
                            o_st = act.tile([64, 256], bf16, tag="ost", bufs=2)
                            nc.vector.tensor_mul(o_st[:], avs[hh][0:64, :],
                                                 bc_sb[0:64, :])
                            nc.sync.dma_start(o_sb[64:128, j, :], o_st[:])
                    # + v bias (valid because sum of softmax weights == 1)
                    nc.vector.tensor_scalar_add(o_sb[:, j, :], o_sb[:, j, :],
                                                qkvb_t[:, 16 + j:16 + j + 1])

                # attention out-proj + residual
                w_proj = []
                for dt in range(ND):
                    wt = wpool.tile([P, 4096], bf16, tag="w", name=f"wproj{l}_{dt}")
                    nc.sync.dma_start(wt[:, 0:D], proj_wT[l, dt * P:(dt + 1) * P, :])
                    w_proj.append(wt)
                projb_t = par.tile([P, ND], f32, tag="lnp")
                nc.sync.dma_start(projb_t[:], proj_b[l].rearrange("(k p) -> p k", p=P))
                for do in range(ND):
                    acc = ps.tile([P, 256], f32, tag="acc")
                    for dt in range(ND):
                        nc.tensor.matmul(acc[:], lhsT=w_proj[dt][:, do * P:(do + 1) * P],
                                         rhs=o_sb[:, dt, :],
                                         start=(dt == 0), stop=(dt == ND - 1))
                    nc.vector.scalar_tensor_tensor(
                        out=x_sb[:, do, :], in0=acc[:], scalar=projb_t[:, do:do + 1],
                        in1=x_sb[:, do, :], op0=OP.add, op1=OP.add)

                # LN2 + MLP
                ln2s_t = par.tile([P, ND], f32, tag="lnp")
                nc.sync.dma_start(ln2s_t[:], ln2_s[l].rearrange("(k p) -> p k", p=P))
                ln2b_t = par.tile([P, ND], f32, tag="lnp")
                nc.sync.dma_start(ln2b_t[:], ln2_b[l].rearrange("(k p) -> p k", p=P))
                h2_sb = act.tile([P, ND, 256], bf16, tag="h", bufs=2)
                _ln(nc, ps, act, rows, cons, x_sb, ln2s_t, ln2b_t, h2_sb)

                w_fc1 = []
                for dt in range(ND):
                    wt = wpool.tile([P, 4096], bf16, tag="w", name=f"wfc1{l}_{dt}")
                    nc.sync.dma_start(wt[:], fc1_wT[l, dt * P:(dt + 1) * P, :])
                    w_fc1.append(wt)
                fc1b_t = par.tile([P, NFT], f32, tag="fcb")
                nc.sync.dma_start(fc1b_t[:], fc1_b[l].rearrange("(k p) -> p k", p=P))
                h1g = act.tile([P, NFT, 256], bf16, tag="h1g")
                for ft in range(NFT):
                    acc = ps.tile([P, 256], f32, tag="acc")
                    for dt in range(ND):
                        nc.tensor.matmul(acc[:], lhsT=w_fc1[dt][:, ft * P:(ft + 1) * P],
                                         rhs=h2_sb[:, dt, :],
                                         start=(dt == 0), stop=(dt == ND - 1))
                    nc.scalar.activation(h1g[:, ft, :], acc[:], AF.Gelu,
                                         bias=fc1b_t[:, ft:ft + 1])

                w_fc2 = []
                for g in range(ND):
                    wt = wpool.tile([P, 4, D], bf16, tag="w", name=f"wfc2{l}_{g}")
                    nc.sync.dma_start(
                        wt[:], fc2_wT[l, g * 512:(g + 1) * 512, :].rearrange(
                            "(i p) d -> p i d", p=P))
                    w_fc2.append(wt)
                fc2b_t = par.tile([P, ND], f32, tag="lnp")
                nc.sync.dma_start(fc2b_t[:], fc2_b[l].rearrange("(k p) -> p k", p=P))
                for do in range(ND):
                    acc = ps.tile([P, 256], f32, tag="acc")
                    for ft in range(NFT):
                        nc.tensor.matmul(
                            acc[:], lhsT=w_fc2[ft // 4][:, ft % 4, do * P:(do + 1) * P],
                            rhs=h1g[:, ft, :],
                            start=(ft == 0), stop=(ft == NFT - 1))
                    nc.vector.scalar_tensor_tensor(
                        out=x_sb[:, do, :], in0=acc[:], scalar=fc2b_t[:, do:do + 1],
                        in1=x_sb[:, do, :], op0=OP.add, op1=OP.add)

            # ---------------- final LN + AllGather + LM head ----------------
            lnfs_t = par.tile([P, ND], f32, tag="lnp")
            nc.sync.dma_start(lnfs_t[:], lnf_s.rearrange("(k p) -> p k", p=P))
            lnfb_t = par.tile([P, ND], f32, tag="lnp")
            nc.sync.dma_start(lnfb_t[:], lnf_b.rearrange("(k p) -> p k", p=P))
            xf_sb = act.tile([P, ND, 256], bf16, tag="h", bufs=2)
            _ln(nc, ps, act, rows, cons, x_sb, lnfs_t, lnfb_t, xf_sb)

            xf_in = dram.tile([ND, P, 256], bf16)
            xf_out = dram.tile([NCORES, ND, P, 256], bf16, addr_space="Shared")
            nc.sync.dma_start(xf_in.rearrange("d p t -> p d t"), xf_sb[:])
            if single:
                nc.sync.dma_start(xf_out[0], xf_in[:])
            else:
                nc.gpsimd.collective_compute(
                    "AllGather", OP.bypass, replica_groups=all_group,
                    ins=[xf_in.opt()], outs=[xf_out.opt()])

            # release layer-phase pools; LM phase gets all 8 PSUM banks
            act.release()
            ps.release()
            lmact = tc.alloc_tile_pool(name="lmact", bufs=1)
            psB = tc.alloc_tile_pool(name="psB", bufs=8, space="PSUM")

            # x (all tokens) as stationary tiles, lm weights as moving rhs
            xall = []
            for g in range(4):
                xt = lmact.tile([P, 2, NT], bf16, tag="xall", bufs=4,
                                name=f"xall{g}")
                for i in range(2):
                    dt = 2 * g + i
                    nc.sync.dma_start(
                        xt[:, i, :].rearrange("p (r t) -> p r t", r=NCORES),
                        xf_out[:, dt, :, :].rearrange("r p t -> p r t"))
                xall.append(xt)
            lmw = []
            for dt in range(ND):
                wt = wpool.tile([P, VSP], bf16, tag="w", name=f"lmw{dt}")
                nc.sync.dma_start(wt[:], lm_wT[dt * P:(dt + 1) * P, :])
                lmw.append(wt)
            # lm bias broadcast to [128, VSP] once (bias varies along the
            # free/vocab axis in token-major layout)
            lmb_row = rows.tile([1, VSP], f32, tag="lmbrow", bufs=1)
            nc.sync.dma_start(lmb_row[:], lm_b[None, :])
            lmb_bc = lmact.tile([P, VSP], f32, tag="lmbbc")
            nc.gpsimd.partition_broadcast(lmb_bc[:], lmb_row[:], channels=P)

            for tk in range(NT // P):
                accs = [psB.tile([P, 512], f32, tag="lmacc", name=f"lmacc{tk}_{v}")
                        for v in range(8)]
                for dt in range(ND):
                    lhs = xall[dt // 2][:, dt % 2, tk * P:(tk + 1) * P]
                    for vc in range(8):
                        nc.tensor.matmul(
                            accs[vc][:], lhsT=lhs,
                            rhs=lmw[dt][:, vc * 512:(vc + 1) * 512],
                            start=(dt == 0), stop=(dt == ND - 1))
                for vc in range(8):
                    osb = lmact.tile([P, 512], f32, tag="osb", bufs=4)
                    nc.vector.tensor_add(osb[:], accs[vc][:],
                                         lmb_bc[:, vc * 512:(vc + 1) * 512])
                    nc.sync.dma_start(
                        out_tok[tk * P:(tk + 1) * P, vc * 512:(vc + 1) * 512],
                        osb[:])
            lmact.release()
            psB.release()

    nc.compile()
    return nc


def _prep_in_maps(inputs, n_layers=L):
    input_ids = np.asarray(inputs["input_ids"]).reshape(NT).astype(np.int32)
    pos_w = np.asarray(inputs["pos_w"], dtype=np.float32)
    embed_w = np.ascontiguousarray(np.asarray(inputs["embed_w"], dtype=np.float32))

    def t_bf(a, perm):
        return np.ascontiguousarray(
            np.transpose(np.asarray(a, dtype=np.float32), perm)
        ).astype(ml_dtypes.bfloat16)

    attn_wT = t_bf(inputs["attn_in_w"], (0, 2, 1))   # [L, D, 3D]
    proj_wT = t_bf(inputs["attn_out_w"], (0, 2, 1))  # [L, D(in), D(out)]
    fc1_wT = t_bf(inputs["fc1_w"], (0, 2, 1))        # [L, D, F]
    fc2_wT = t_bf(inputs["fc2_w"], (0, 2, 1))        # [L, F, D]

    lm_w = np.asarray(inputs["lm_w"], dtype=np.float32)
    lm_b_full = np.asarray(inputs["lm_b"], dtype=np.float32)

    common = {
        "embed_w": embed_w,
        "attn_wT": attn_wT,
        "attn_in_b": np.asarray(inputs["attn_in_b"], dtype=np.float32),
        "proj_wT": proj_wT,
        "proj_b": np.asarray(inputs["attn_out_b"], dtype=np.float32),
        "fc1_wT": fc1_wT,
        "fc1_b": np.asarray(inputs["fc1_b"], dtype=np.float32),
        "fc2_wT": fc2_wT,
        "fc2_b": np.asarray(inputs["fc2_b"], dtype=np.float32),
        "ln1_s": np.asarray(inputs["ln1_s"], dtype=np.float32),
        "ln1_b": np.asarray(inputs["ln1_b"], dtype=np.float32),
        "ln2_s": np.asarray(inputs["ln2_s"], dtype=np.float32),
        "ln2_b": np.asarray(inputs["ln2_b"], dtype=np.float32),
        "lnf_s": np.asarray(inputs["lnf_s"], dtype=np.float32),
        "lnf_b": np.asarray(inputs["lnf_b"], dtype=np.float32),
    }

    in_maps = []
    for c in range(NCORES):
        s0 = (c % 4) * T
        lm_shard = np.zeros((VSP, D), np.float32)
        lm_shard[:VS] = lm_w[c * VS:(c + 1) * VS]
        lmb_shard = np.zeros(VSP, np.float32)
        lmb_shard[:VS] = lm_b_full[c * VS:(c + 1) * VS]
        m = dict(common)
        m["ids"] = input_ids[c * T:(c + 1) * T]
        m["pos"] = np.ascontiguousarray(pos_w[s0:s0 + T])
        m["lm_wT"] = np.ascontiguousarray(lm_shard.T).astype(ml_dtypes.bfloat16)
        m["lm_b"] = lmb_shard
        in_maps.append(m)
    return in_maps


def _assemble(results):
    parts = [results[c]["out_tok"][:, :VS] for c in range(NCORES)]
    logits = np.concatenate(parts, axis=1)     # [2048, 32000]
    return np.ascontiguousarray(logits.reshape(B, S, V).astype(np.float32))


_NC_CACHE = {}


def _get_nc(n_layers=L):
    if n_layers not in _NC_CACHE:
        _NC_CACHE[n_layers] = build(n_layers)
    return _NC_CACHE[n_layers]


def run(inputs, n_layers=L, trace=False, trace_cores=None):
    nc = _get_nc(n_layers)
    in_maps = _prep_in_maps(inputs, n_layers)
    if trace:
        try:
            import axon_ntff_shim
            axon_ntff_shim.install()
        except Exception:
            pass
    res = bass_utils.run_bass_kernel_spmd(
        nc, in_maps, core_ids=list(range(NCORES)), trace=trace,
        trace_cores=(trace_cores or [0]) if trace else None)
    return _assemble(res.results), res


def kernel(**inputs) -> np.ndarray:
    out, _ = run(inputs)
    return out



# revision 6
# speedup vs baseline: 1.1375x; 1.1375x over previous
"""Trainium2 Bass kernel for a 4-layer transformer (B=2,S=1024,D=1024,H=16,F=4096,V=32000).

Strategy (8 NeuronCores):
 - Sequence-parallel transformer layers: each core owns 256 tokens
   (cores 0-3: batch 0, cores 4-7: batch 1). All layer weights replicated
   (streamed from HBM as bf16). Activations kept feature-major
   ([d on partitions, tokens on free]) so no transposes are needed in the
   layer loop; per-token stats (LN mean/var, softmax 1/Z) are computed with
   ones-matmuls and broadcast back.
 - LayerNorm scale/bias are folded into the following projection weights
   host-side (exact), so the device LN is just (x-mu)*rsqrt(var+eps), with
   stats from a single bf16 [x|x^2] matmul chain.
 - Attention: per-layer AllGather of K/V (bf16) within each batch's 4-core
   group; scores computed transposed (s^T[kt,q]) so softmax-weighted sums
   contract on the partition axis without transposing P.
 - LM head: each core computes a 4000-wide vocab shard for all 2048 tokens;
   its OWN 256-token block is computed directly from local activations
   (overlapping the final AllGather), then the other 7 blocks in ring order.
   Output written in local block order; host reassembles.
 - A tiny warmup AllGather at kernel start absorbs cross-core dispatch skew
   so the first real collective runs at full ring speed.

Self-contained: hardcodes all shapes; host side only reshapes/shards/casts.
"""
import numpy as np
import ml_dtypes

import concourse.bass as bass
import concourse.bacc as bacc
import concourse.mybir as mybir
import concourse.tile as tile
from concourse import bass_utils
from concourse.masks import make_identity

B, S, D, H, L, F, V = 2, 1024, 1024, 16, 4, 4096, 32000
DH = D // H          # 64
NCORES = 8
T = (B * S) // NCORES  # 256 tokens per core
NT = B * S             # 2048
VS = V // NCORES       # 4000
VSP = 4096             # padded vocab shard
P = 128
ND = D // P            # 8 d-tiles
NFT = F // P           # 32 fc1 f-tiles

f32 = mybir.dt.float32
bf16 = mybir.dt.bfloat16
i32 = mybir.dt.int32
AF = mybir.ActivationFunctionType
OP = mybir.AluOpType


def _ln(nc, ps, act, rows, cons, x_sb, out_h):
    """LayerNorm (scale/bias folded into downstream weights):
    x_sb [128, 8, 256] f32 -> out_h = (x-mu)*rsqrt(var+eps) [128,8,256] bf16.

    Stats via one bf16 matmul chain: per d-tile build [x|x^2] bf16 [128,512],
    accumulate ones(1/D)-matmul -> stat[1,512] = [mu | E[x^2]]."""
    stat = ps.tile([1, 512], f32, tag="att_s", bufs=4)
    for dt in range(ND):
        xx = act.tile([P, 512], bf16, tag="sq", bufs=3)
        nc.scalar.activation(xx[:, 0:256], x_sb[:, dt, :], AF.Copy)
        nc.vector.tensor_mul(xx[:, 256:512], x_sb[:, dt, :], x_sb[:, dt, :])
        nc.tensor.matmul(stat[:], lhsT=cons.ones_col[:], rhs=xx[:],
                         start=(dt == 0), stop=(dt == ND - 1))
    murow = rows.tile([1, 512], f32, tag="row")  # [mu | invstd]
    nc.scalar.activation(murow[:, 0:256], stat[:, 0:256], AF.Copy)
    mu2 = rows.tile([1, 256], f32, tag="row")
    nc.vector.tensor_mul(mu2[:], murow[:, 0:256], murow[:, 0:256])
    var = rows.tile([1, 256], f32, tag="row")
    nc.vector.tensor_sub(var[:], stat[:, 256:512], mu2[:])
    std = rows.tile([1, 256], f32, tag="row")
    nc.scalar.activation(std[:], var[:], AF.Sqrt, bias=cons.eps_row[:, 0:1])
    nc.vector.reciprocal_approx_fast(murow[:, 256:512], std[:])
    bc = act.tile([P, 512], f32, tag="lnbc", bufs=2)
    nc.gpsimd.partition_broadcast(bc[:], murow[:], channels=P)
    for dt in range(ND):
        t = act.tile([P, 256], f32, tag="sq", bufs=3)
        nc.vector.tensor_sub(t[:], x_sb[:, dt, :], bc[:, 0:256])
        nc.vector.tensor_mul(out_h[:, dt, :], t[:], bc[:, 256:512])


class _Cons:
    pass


def build(n_layers=L, single=False):
    """single=True: 1-core variant with collectives replaced by local DMA
    copies (for TimelineSim cost-model analysis only — wrong numerics)."""
    nc = bacc.Bacc("TRN2", target_bir_lowering=False, debug=False,
                   num_devices=1 if single else NCORES)

    ids = nc.dram_tensor("ids", [T], i32, kind="ExternalInput").ap()
    pos = nc.dram_tensor("pos", [T, D], f32, kind="ExternalInput").ap()
    embed_w = nc.dram_tensor("embed_w", [V, D], f32, kind="ExternalInput").ap()
    attn_wT = nc.dram_tensor("attn_wT", [L, D, 3 * D], bf16, kind="ExternalInput").ap()
    attn_in_b = nc.dram_tensor("attn_in_b", [L, 3 * D], f32, kind="ExternalInput").ap()
    proj_wT = nc.dram_tensor("proj_wT", [L, D, D], bf16, kind="ExternalInput").ap()
    proj_b = nc.dram_tensor("proj_b", [L, D], f32, kind="ExternalInput").ap()
    fc1_wT = nc.dram_tensor("fc1_wT", [L, D, F], bf16, kind="ExternalInput").ap()
    fc1_b = nc.dram_tensor("fc1_b", [L, F], f32, kind="ExternalInput").ap()
    fc2_wT = nc.dram_tensor("fc2_wT", [L, F, D], bf16, kind="ExternalInput").ap()
    fc2_b = nc.dram_tensor("fc2_b", [L, D], f32, kind="ExternalInput").ap()
    lm_wT = nc.dram_tensor("lm_wT", [D, VSP], bf16, kind="ExternalInput").ap()
    lm_b = nc.dram_tensor("lm_b", [VSP], f32, kind="ExternalInput").ap()
    out_tok = nc.dram_tensor("out_tok", [16, P, VSP], f32, kind="ExternalOutput").ap()

    kv_groups = [[0, 1, 2, 3], [4, 5, 6, 7]]
    all_group = [list(range(NCORES))]

    with tile.TileContext(nc) as tc:
        with (
            tc.tile_pool(name="consp", bufs=1) as consp,
            tc.tile_pool(name="wpool", bufs=9) as wpool,
            tc.tile_pool(name="rows", bufs=6) as rows,
            tc.tile_pool(name="par", bufs=2) as par,
            tc.tile_pool(name="dram", bufs=1, space="DRAM") as dram,
        ):
            # layer-phase pools, released before the LM phase so the LM
            # phase can use all 8 PSUM banks and the freed SBUF
            act = tc.alloc_tile_pool(name="act", bufs=1)
            ps = tc.alloc_tile_pool(name="ps", bufs=2, space="PSUM")

            # warmup collective: absorbs cross-core dispatch skew and warms
            # the CC rings so layer-0's K AllGather runs at full speed
            wu_sb = par.tile([1, 64], bf16, tag="wu")
            nc.vector.memset(wu_sb[:], 0.0)
            wu_in = dram.tile([64], bf16, name="wu_in")
            wu_out = dram.tile([NCORES, 64], bf16, name="wu_out")
            nc.sync.dma_start(wu_in[None, :], wu_sb[:])
            if single:
                nc.sync.dma_start(wu_out[0][None, :], wu_in[None, :])
            else:
                nc.gpsimd.collective_compute(
                    "AllGather", OP.bypass, replica_groups=all_group,
                    ins=[wu_in.opt()], outs=[wu_out.opt()])

            cons = _Cons()
            ident = consp.tile([P, P], f32)
            make_identity(nc, ident)
            ident_bf = consp.tile([P, P], bf16)
            nc.vector.tensor_copy(ident_bf[:], ident[:])
            ones_col = consp.tile([P, 1], bf16)
            nc.vector.memset(ones_col[:], 1.0 / D)
            cons.ones_col = ones_col
            eps_row = consp.tile([1, 1], f32)
            nc.vector.memset(eps_row[:], 1e-5)
            cons.eps_row = eps_row

            x_sb = consp.tile([P, ND, 256], f32)  # residual, feature-major
            xf_sb = consp.tile([P, ND, 256], bf16)  # final-LN out (LM lhsT)

            # per-core group-rank registers for own-block-skipping dynamic DMAs
            seng = nc.sync
            pid = seng.partition_id()
            rgrp = seng.alloc_register("grp_rank")
            seng.reg_alu(rgrp, pid, 3, OP.bitwise_and)
            grp_rank = seng.snap(rgrp, donate=True, min_val=0, max_val=3)
            oth_ranks = []
            for i in range(3):
                ra = seng.alloc_register(f"oth{i}a")
                seng.reg_alu(ra, grp_rank, i + 1, OP.add)
                rb = seng.alloc_register(f"oth{i}b")
                seng.reg_alu(rb, ra, 3, OP.bitwise_and)
                oth_ranks.append(seng.snap(rb, donate=True, min_val=0, max_val=3))
            # global-rank rotation for the LM phase (other 7 cores)
            oth_g = []
            for i in range(7):
                ra = seng.alloc_register(f"og{i}a")
                seng.reg_alu(ra, pid, i + 1, OP.add)
                rb = seng.alloc_register(f"og{i}b")
                seng.reg_alu(rb, ra, 7, OP.bitwise_and)
                oth_g.append(seng.snap(rb, donate=True, min_val=0, max_val=7))

            # ---------------- embedding ----------------
            for tc2 in range(2):
                ids_sb = par.tile([P, 1], i32, tag="ids")
                nc.sync.dma_start(ids_sb[:], ids[tc2 * P:(tc2 + 1) * P, None])
                gat = wpool.tile([P, D], f32, tag="w")
                nc.gpsimd.indirect_dma_start(
                    out=gat[:], out_offset=None, in_=embed_w[:],
                    in_offset=bass.IndirectOffsetOnAxis(ap=ids_sb[:, :1], axis=0))
                pos_sb = wpool.tile([P, D], f32, tag="w")
                nc.sync.dma_start(pos_sb[:], pos[tc2 * P:(tc2 + 1) * P, :])
                nc.vector.tensor_add(gat[:], gat[:], pos_sb[:])
                for dt in range(ND):
                    tp = ps.tile([P, P], f32, tag="att_s", bufs=4)
                    nc.tensor.transpose(tp[:], gat[:, dt * P:(dt + 1) * P], ident[:])
                    nc.vector.tensor_copy(x_sb[:, dt, tc2 * P:(tc2 + 1) * P], tp[:])

            # ---------------- layers ----------------
            for l in range(n_layers):
                # LN1 (scale/bias folded into attn_wT/attn_in_b host-side)
                h_sb = act.tile([P, ND, 256], bf16, tag="h", bufs=2)
                _ln(nc, ps, act, rows, cons, x_sb, h_sb)

                # QKV weights: 8 d-slices of [128, 3072]
                w_qkv = []
                for dt in range(ND):
                    wt = wpool.tile([P, 4096], bf16, tag="w", name=f"wqkv{l}_{dt}")
                    nc.sync.dma_start(wt[:, 0:3 * D], attn_wT[l, dt * P:(dt + 1) * P, :])
                    w_qkv.append(wt)
                qkvb_t = par.tile([P, 24], f32, tag="qkvb")
                nc.sync.dma_start(qkvb_t[:], attn_in_b[l].rearrange("(k p) -> p k", p=P))

                q_all = act.tile([P, 8, 256], bf16, tag="q")
                k_loc = act.tile([P, 8, 256], bf16, tag="kloc")
                # K first so the AllGather can start as early as possible
                for ft in range(8, 16):
                    acc = ps.tile([P, 256], f32, tag="acc")
                    for dt in range(ND):
                        nc.tensor.matmul(acc[:], lhsT=w_qkv[dt][:, ft * P:(ft + 1) * P],
                                         rhs=h_sb[:, dt, :],
                                         start=(dt == 0), stop=(dt == ND - 1))
                    nc.scalar.activation(k_loc[:, ft - 8, :], acc[:], AF.Identity,
                                         bias=qkvb_t[:, ft:ft + 1])
                # K bounce + AllGather immediately (overlaps V/Q projections)
                k_in = dram.tile([8, P, 256], bf16, tag="kin", name=f"kin{l}")
                k_out = dram.tile([4, 8, P, 256], bf16, tag="kout", name=f"kout{l}")
                nc.sync.dma_start(k_in.rearrange("f p t -> p f t"), k_loc[:])
                if single:
                    nc.sync.dma_start(k_out[0], k_in[:])
                else:
                    nc.gpsimd.collective_compute(
                        "AllGather", OP.bypass, replica_groups=kv_groups,
                        ins=[k_in.opt()], outs=[k_out.opt()])

                # V (token-major with interleaved ones columns: per head 65
                # cols = [v_h | 1]), then V AllGather in that layout
                v_loc = act.tile([P, 2, 16 * 65], bf16, tag="vloc")
                v_loc_h = v_loc.rearrange("p c (h g) -> p c h g", h=16, g=65)
                for tc2 in range(2):
                    for nb in range(2):
                        acc = ps.tile([P, 512], f32, tag="acc")
                        for dt in range(ND):
                            nc.tensor.matmul(
                                acc[:], lhsT=h_sb[:, dt, tc2 * P:(tc2 + 1) * P],
                                rhs=w_qkv[dt][:, 2 * D + nb * 512:2 * D + (nb + 1) * 512],
                                start=(dt == 0), stop=(dt == ND - 1))
                        nc.scalar.activation(
                            v_loc_h[:, tc2, nb * 8:(nb + 1) * 8, 0:64],
                            acc[:].rearrange("p (h g) -> p h g", h=8), AF.Copy)
                    nc.vector.memset(v_loc_h[:, tc2, :, 64:65], 1.0)
                v_in = dram.tile([256, 16 * 65], bf16, tag="vin", name=f"vin{l}")
                v_out = dram.tile([4, 256, 16 * 65], bf16, tag="vout", name=f"vout{l}")
                for tc2 in range(2):
                    nc.sync.dma_start(v_in[tc2 * P:(tc2 + 1) * P, :],
                                      v_loc[:, tc2, :])
                if single:
                    nc.sync.dma_start(v_out[0], v_in[:])
                else:
                    nc.gpsimd.collective_compute(
                        "AllGather", OP.bypass, replica_groups=kv_groups,
                        ins=[v_in.opt()], outs=[v_out.opt()])

                # Q projections (overlap the AllGathers)
                for ft in range(8):
                    acc = ps.tile([P, 256], f32, tag="acc")
                    for dt in range(ND):
                        nc.tensor.matmul(acc[:], lhsT=w_qkv[dt][:, ft * P:(ft + 1) * P],
                                         rhs=h_sb[:, dt, :],
                                         start=(dt == 0), stop=(dt == ND - 1))
                    nc.scalar.activation(q_all[:, ft, :], acc[:], AF.Identity,
                                         bias=qkvb_t[:, ft:ft + 1])

                o_sb = act.tile([P, ND, 256], bf16, tag="o")
                scale = 1.0 / np.sqrt(DH)

                # Pass 1 (pre-AllGather): attention over this core's OWN 256
                # k-tokens from local k_loc/v_loc; partial [o|Z] snapshotted
                # to SBUF so the PSUM slot frees immediately.
                o_own = {}
                for j in range(8):
                    for hh in range(2):
                        h_idx = 2 * j + hh
                        base = hh * 64
                        avp = ps.tile([P, 256], f32, tag="av",
                                      name=f"avp{l}_{j}_{hh}")
                        for c in range(2):
                            sps = ps.tile([P, 256], f32, tag="att_s", bufs=4,
                                          name=f"spp{l}_{j}_{c}_{hh}")
                            nc.tensor.matmul(
                                sps[:],
                                lhsT=k_loc[base:base + 64, j, c * P:(c + 1) * P],
                                rhs=q_all[base:base + 64, j, :],
                                start=True, stop=True)
                            e = act.tile([P, 256], bf16, tag="e", bufs=8,
                                         name=f"ep{l}_{j}_{c}_{hh}")
                            nc.scalar.activation(e[:], sps[:], AF.Exp, scale=scale)
                            nc.tensor.matmul(
                                avp[0:65, :],
                                lhsT=v_loc_h[:, c, h_idx, :],
                                rhs=e[:], start=(c == 0), stop=(c == 1))
                        snap = act.tile([65, 256], bf16, tag="avown", bufs=16,
                                        name=f"oo{l}_{j}_{hh}")
                        nc.scalar.activation(snap[:], avp[0:65, :], AF.Copy)
                        o_own[(j, hh)] = snap

                # Load the three OTHER ranks' K/V blocks (partition-id-derived
                # dynamic offsets skip our own block).
                k_sb = act.tile([P, 8, 768], bf16, tag="ksb")
                for j in range(8):
                    for i in range(3):
                        nc.sync.dma_start(
                            k_sb[:, j, i * 256:(i + 1) * 256],
                            k_out[bass.ds(oth_ranks[i], 1), j, :, :].rearrange(
                                "o p t -> p (o t)"))
                v_sb = act.tile([P, 6, 16 * 65], bf16, tag="vsb")
                for i in range(3):
                    nc.sync.dma_start(
                        v_sb[:, 2 * i:2 * i + 2, :],
                        v_out[bass.ds(oth_ranks[i], 1), :, :].rearrange(
                            "o (th p) f -> p (o th) f", p=P))
                v_sb_h = v_sb.rearrange("p c (h g) -> p c h g", h=16, g=65)

                # Pass 2: re-inject the partial [o|Z] (identity matmul) and
                # accumulate the 6 remaining k-chunks; head pairs interleaved
                # so LDWEIGHTS of one head overlaps the matmul of the other.
                for j in range(8):
                    av0 = ps.tile([P, 256], f32, tag="av", name=f"av{l}_{j}_0")
                    av1 = ps.tile([P, 256], f32, tag="av", name=f"av{l}_{j}_1")
                    avs = [av0, av1]
                    for hh in range(2):
                        nc.tensor.matmul(avs[hh][0:65, :],
                                         lhsT=ident_bf[0:65, 0:65],
                                         rhs=o_own[(j, hh)][:],
                                         start=True, stop=False)
                    for c in range(6):
                        es = []
                        for hh in range(2):
                            base = hh * 64
                            sps = ps.tile([P, 256], f32, tag="att_s", bufs=4,
                                          name=f"sps{l}_{j}_{c}_{hh}")
                            nc.tensor.matmul(
                                sps[:], lhsT=k_sb[base:base + 64, j, c * P:(c + 1) * P],
                                rhs=q_all[base:base + 64, j, :], start=True, stop=True)
                            e = act.tile([P, 256], bf16, tag="e", bufs=8,
                                         name=f"e{l}_{j}_{c}_{hh}")
                            nc.scalar.activation(e[:], sps[:], AF.Exp, scale=scale)
                            es.append(e)
                        for hh in range(2):
                            h_idx = 2 * j + hh
                            # av rows 0:64 = unnormalized o, row 64 = Z
                            nc.tensor.matmul(
                                avs[hh][0:65, :],
                                lhsT=v_sb_h[:, c, h_idx, :],
                                rhs=es[hh][:], start=False, stop=(c == 5))
                    for hh in range(2):
                        # stage Z to SBUF: reciprocal_approx_fast's fp32
                        # bit-trick seed must not read raw PSUM bits
                        zrow = rows.tile([1, 256], f32, tag="row")
                        nc.scalar.activation(zrow[:], avs[hh][64:65, :], AF.Copy)
                        recip = rows.tile([1, 256], f32, tag="row")
                        nc.vector.reciprocal_approx_fast(recip[:], zrow[:])
                        bc_sb = act.tile([P, 256], f32, tag="bcsb", bufs=2)
                        nc.gpsimd.partition_broadcast(bc_sb[0:64, :], recip[:],
                                                      channels=64)
                        if hh == 0:
                            nc.vector.tensor_mul(o_sb[0:64, j, :], avs[hh][0:64, :],
                                                 bc_sb[0:64, :])
                        else:
                            o_st = act.tile([64, 256], bf16, tag="ost", bufs=2)
                            nc.vector.tensor_mul(o_st[:], avs[hh][0:64, :],
                                                 bc_sb[0:64, :])
                            nc.sync.dma_start(o_sb[64:128, j, :], o_st[:])
                    # + v bias (valid because sum of softmax weights == 1)
                    nc.vector.tensor_scalar_add(o_sb[:, j, :], o_sb[:, j, :],
                                                qkvb_t[:, 16 + j:16 + j + 1])

                # attention out-proj + residual
                w_proj = []
                for dt in range(ND):
                    wt = wpool.tile([P, 4096], bf16, tag="w", name=f"wproj{l}_{dt}")
                    nc.sync.dma_start(wt[:, 0:D], proj_wT[l, dt * P:(dt + 1) * P, :])
                    w_proj.append(wt)
                projb_t = par.tile([P, ND], f32, tag="lnp")
                nc.sync.dma_start(projb_t[:], proj_b[l].rearrange("(k p) -> p k", p=P))
                for do in range(ND):
                    acc = ps.tile([P, 256], f32, tag="acc")
                    for dt in range(ND):
                        nc.tensor.matmul(acc[:], lhsT=w_proj[dt][:, do * P:(do + 1) * P],
                                         rhs=o_sb[:, dt, :],
                                         start=(dt == 0), stop=(dt == ND - 1))
                    nc.vector.scalar_tensor_tensor(
                        out=x_sb[:, do, :], in0=acc[:], scalar=projb_t[:, do:do + 1],
                        in1=x_sb[:, do, :], op0=OP.add, op1=OP.add)

                # LN2 + MLP (ln2 scale/bias folded into fc1_wT/fc1_b)
                h2_sb = act.tile([P, ND, 256], bf16, tag="h", bufs=2)
                _ln(nc, ps, act, rows, cons, x_sb, h2_sb)

                w_fc1 = []
                for dt in range(ND):
                    wt = wpool.tile([P, 4096], bf16, tag="w", name=f"wfc1{l}_{dt}")
                    nc.sync.dma_start(wt[:], fc1_wT[l, dt * P:(dt + 1) * P, :])
                    w_fc1.append(wt)
                fc1b_t = par.tile([P, NFT], f32, tag="fcb")
                nc.sync.dma_start(fc1b_t[:], fc1_b[l].rearrange("(k p) -> p k", p=P))
                h1g = act.tile([P, NFT, 256], bf16, tag="h1g")
                for ft in range(NFT):
                    acc = ps.tile([P, 256], f32, tag="acc")
                    for dt in range(ND):
                        nc.tensor.matmul(acc[:], lhsT=w_fc1[dt][:, ft * P:(ft + 1) * P],
                                         rhs=h2_sb[:, dt, :],
                                         start=(dt == 0), stop=(dt == ND - 1))
                    nc.scalar.activation(h1g[:, ft, :], acc[:], AF.Gelu,
                                         bias=fc1b_t[:, ft:ft + 1])

                w_fc2 = []
                for g in range(ND):
                    wt = wpool.tile([P, 4, D], bf16, tag="w", name=f"wfc2{l}_{g}")
                    nc.sync.dma_start(
                        wt[:], fc2_wT[l, g * 512:(g + 1) * 512, :].rearrange(
                            "(i p) d -> p i d", p=P))
                    w_fc2.append(wt)
                fc2b_t = par.tile([P, ND], f32, tag="lnp")
                nc.sync.dma_start(fc2b_t[:], fc2_b[l].rearrange("(k p) -> p k", p=P))
                if l == n_layers - 1:
                    # prefetch LM-head weights while the last MLP runs
                    lmw = []
                    for dt in range(ND):
                        wt = wpool.tile([P, VSP], bf16, tag="w", name=f"lmw{dt}")
                        nc.sync.dma_start(wt[:], lm_wT[dt * P:(dt + 1) * P, :])
                        lmw.append(wt)
                    lmb_row = rows.tile([1, VSP], f32, tag="lmbrow", bufs=1)
                    nc.sync.dma_start(lmb_row[:], lm_b[None, :])
                for do in range(ND):
                    acc = ps.tile([P, 256], f32, tag="acc")
                    for ft in range(NFT):
                        nc.tensor.matmul(
                            acc[:], lhsT=w_fc2[ft // 4][:, ft % 4, do * P:(do + 1) * P],
                            rhs=h1g[:, ft, :],
                            start=(ft == 0), stop=(ft == NFT - 1))
                    nc.vector.scalar_tensor_tensor(
                        out=x_sb[:, do, :], in0=acc[:], scalar=fc2b_t[:, do:do + 1],
                        in1=x_sb[:, do, :], op0=OP.add, op1=OP.add)

            if n_layers == 0:
                lmw = []
                for dt in range(ND):
                    wt = wpool.tile([P, VSP], bf16, tag="w", name=f"lmw{dt}")
                    nc.sync.dma_start(wt[:], lm_wT[dt * P:(dt + 1) * P, :])
                    lmw.append(wt)
                lmb_row = rows.tile([1, VSP], f32, tag="lmbrow", bufs=1)
                nc.sync.dma_start(lmb_row[:], lm_b[None, :])

            # ------------- final LN + AllGather + LM head -------------
            # (lnf scale/bias folded into lm_wT/lm_b host-side)
            _ln(nc, ps, act, rows, cons, x_sb, xf_sb)

            xf_in = dram.tile([ND, P, 256], bf16)
            xf_out = dram.tile([NCORES, ND, P, 256], bf16, addr_space="Shared")
            nc.sync.dma_start(xf_in.rearrange("d p t -> p d t"), xf_sb[:])
            if single:
                nc.sync.dma_start(xf_out[0], xf_in[:])
            else:
                nc.gpsimd.collective_compute(
                    "AllGather", OP.bypass, replica_groups=all_group,
                    ins=[xf_in.opt()], outs=[xf_out.opt()])

            # release layer-phase pools; LM phase gets all 8 PSUM banks
            act.release()
            ps.release()
            lmact = tc.alloc_tile_pool(name="lmact", bufs=1)
            psB = tc.alloc_tile_pool(name="psB", bufs=8, space="PSUM")

            # lm bias broadcast to [128, VSP] once (bias varies along the
            # free/vocab axis in token-major layout)
            lmb_bc = lmact.tile([P, VSP], f32, tag="lmbbc")
            nc.gpsimd.partition_broadcast(lmb_bc[:], lmb_row[:], channels=P)

            def lm_block(b, lhs_tile):
                """One 128-token output block: lhs_tile(dt) -> [128,128] bf16."""
                accs = [psB.tile([P, 512], f32, tag="lmacc", name=f"lmacc{b}_{v}")
                        for v in range(8)]
                for dt in range(ND):
                    lhs = lhs_tile(dt)
                    for vc in range(8):
                        nc.tensor.matmul(
                            accs[vc][:], lhsT=lhs,
                            rhs=lmw[dt][:, vc * 512:(vc + 1) * 512],
                            start=(dt == 0), stop=(dt == ND - 1))
                for vc in range(8):
                    osb = lmact.tile([P, 512], f32, tag="osb", bufs=4)
                    nc.vector.tensor_add(osb[:], accs[vc][:],
                                         lmb_bc[:, vc * 512:(vc + 1) * 512])
                    nc.sync.dma_start(
                        out_tok[b, :, vc * 512:(vc + 1) * 512], osb[:])

            # own 256 tokens first — needs only xf_sb, overlaps the AllGather
            for tk2 in range(2):
                lm_block(tk2, lambda dt, tk2=tk2: xf_sb[:, dt, tk2 * P:(tk2 + 1) * P])

            # other 7 ranks' blocks in ring order (dynamic read offsets)
            for i in range(7):
                xo = lmact.tile([P, ND, 256], bf16, tag="xo", bufs=3,
                                name=f"xo{i}")
                nc.sync.dma_start(
                    xo[:],
                    xf_out[bass.ds(oth_g[i], 1), :, :, :].rearrange(
                        "o d p t -> p (o d) t"))
                for tk2 in range(2):
                    lm_block(2 + 2 * i + tk2,
                             lambda dt, xo=xo, tk2=tk2: xo[:, dt, tk2 * P:(tk2 + 1) * P])

            lmact.release()
            psB.release()

    nc.compile()
    return nc


def _prep_in_maps(inputs, n_layers=L):
    input_ids = np.asarray(inputs["input_ids"]).reshape(NT).astype(np.int32)
    pos_w = np.asarray(inputs["pos_w"], dtype=np.float32)
    embed_w = np.ascontiguousarray(np.asarray(inputs["embed_w"], dtype=np.float32))

    # fold LN scale/bias into the downstream projections (exact):
    #   h = (x-mu)*inv*s + b ; W@h + c == (W*s)@((x-mu)*inv) + (c + W@b)
    ln1_s = np.asarray(inputs["ln1_s"], np.float32)
    ln1_b = np.asarray(inputs["ln1_b"], np.float32)
    ln2_s = np.asarray(inputs["ln2_s"], np.float32)
    ln2_b = np.asarray(inputs["ln2_b"], np.float32)
    lnf_s = np.asarray(inputs["lnf_s"], np.float32)
    lnf_b = np.asarray(inputs["lnf_b"], np.float32)
    attn_in_w = np.asarray(inputs["attn_in_w"], np.float32)   # [L,3D,D]
    fc1_w = np.asarray(inputs["fc1_w"], np.float32)           # [L,F,D]
    lm_w = np.asarray(inputs["lm_w"], np.float32)             # [V,D]

    attn_w_eff = attn_in_w * ln1_s[:, None, :]
    attn_b_eff = (np.asarray(inputs["attn_in_b"], np.float32)
                  + np.einsum("led,ld->le", attn_in_w, ln1_b))
    fc1_w_eff = fc1_w * ln2_s[:, None, :]
    fc1_b_eff = (np.asarray(inputs["fc1_b"], np.float32)
                 + np.einsum("lfd,ld->lf", fc1_w, ln2_b))
    lm_w_eff = lm_w * lnf_s[None, :]
    lm_b_eff = np.asarray(inputs["lm_b"], np.float32) + lm_w @ lnf_b

    def t_bf(a, perm):
        return np.ascontiguousarray(
            np.transpose(np.asarray(a, dtype=np.float32), perm)
        ).astype(ml_dtypes.bfloat16)

    attn_wT = t_bf(attn_w_eff, (0, 2, 1))            # [L, D, 3D]
    proj_wT = t_bf(inputs["attn_out_w"], (0, 2, 1))  # [L, D(in), D(out)]
    fc1_wT = t_bf(fc1_w_eff, (0, 2, 1))              # [L, D, F]
    fc2_wT = t_bf(inputs["fc2_w"], (0, 2, 1))        # [L, F, D]

    common = {
        "embed_w": embed_w,
        "attn_wT": attn_wT,
        "attn_in_b": attn_b_eff,
        "proj_wT": proj_wT,
        "proj_b": np.asarray(inputs["attn_out_b"], dtype=np.float32),
        "fc1_wT": fc1_wT,
        "fc1_b": fc1_b_eff,
        "fc2_wT": fc2_wT,
        "fc2_b": np.asarray(inputs["fc2_b"], dtype=np.float32),
    }

    in_maps = []
    for c in range(NCORES):
        s0 = (c % 4) * T
        lm_shard = np.zeros((VSP, D), np.float32)
        lm_shard[:VS] = lm_w_eff[c * VS:(c + 1) * VS]
        lmb_shard = np.zeros(VSP, np.float32)
        lmb_shard[:VS] = lm_b_eff[c * VS:(c + 1) * VS]
        m = dict(common)
        m["ids"] = input_ids[c * T:(c + 1) * T]
        m["pos"] = np.ascontiguousarray(pos_w[s0:s0 + T])
        m["lm_wT"] = np.ascontiguousarray(lm_shard.T).astype(ml_dtypes.bfloat16)
        m["lm_b"] = lmb_shard
        in_maps.append(m)
    return in_maps


def _assemble(results):
    logits = np.empty((NT, V), np.float32)
    for c in range(NCORES):
        blocks = results[c]["out_tok"]  # [16, 128, VSP] in local block order
        for b in range(16):
            if b < 2:
                r, j = c, b
            else:
                i, j = divmod(b - 2, 2)
                r = (c + 1 + i) % NCORES
            rows = slice(r * T + j * P, r * T + (j + 1) * P)
            logits[rows, c * VS:(c + 1) * VS] = blocks[b, :, :VS]
    return np.ascontiguousarray(logits.reshape(B, S, V).astype(np.float32))


_NC_CACHE = {}


def _get_nc(n_layers=L):
    if n_layers not in _NC_CACHE:
        _NC_CACHE[n_layers] = build(n_layers)
    return _NC_CACHE[n_layers]


def run(inputs, n_layers=L, trace=False, trace_cores=None):
    nc = _get_nc(n_layers)
    in_maps = _prep_in_maps(inputs, n_layers)
    if trace:
        try:
            import axon_ntff_shim
            axon_ntff_shim.install()
        except Exception:
            pass
    res = bass_utils.run_bass_kernel_spmd(
        nc, in_maps, core_ids=list(range(NCORES)), trace=trace,
        trace_cores=(trace_cores or [0]) if trace else None)
    return _assemble(res.results), res


def kernel(**inputs) -> np.ndarray:
    out, _ = run(inputs)
    return out
